# revision 1
# baseline (speedup 1.0000x reference)
"""BiMapGeo forward on 8 NeuronCores (TRN2, Bass/Tile).

P[b,o] = sum_c W[o,c]^T @ x[b,c] @ W[o,c]
  x: (256, 8, 128, 128) fp32 (symmetric in last two dims)
  W: (16, 8, 128, 64) fp32
  P: (256, 16, 64, 64) fp32

Sharding: data-parallel over batch (32 per core), W replicated.

Per-core kernel structure (per group of BG=4 batches):
  mm1: M1[b,c] = x[b,c] @ Wc           (fp32r, stationary=x[b,c] via symmetry,
                                        moving=W[:,c] as [128, 1024] in 2x512)
  evict: PSUM fp32 -> SBUF bf16        (VectorE / ScalarE alternating)
  mm2: P[b,o] += W[o,c]^T @ M1[b,o,c]  (bf16, col-tiled o-pairs, N=256,
                                        accumulate over c in PSUM fp32)
  evict P -> SBUF -> DMA out
"""

import numpy as np
from contextlib import ExitStack

import concourse.bacc as bacc
import concourse.tile as tile
from concourse import mybir

B_TOT, HI, HO, NI, NO = 256, 8, 16, 128, 64
NCORES = 8
B = B_TOT // NCORES  # 32 batches per core
BG = 4               # batches per group
NG = B // BG         # 8 groups
OQ = HO * NO         # 1024

F32 = mybir.dt.float32
F32R = mybir.dt.float32r
BF16 = mybir.dt.bfloat16

_NC_CACHE = {}


def build_nc(loop_iters: int = 1):
    nc = bacc.Bacc("TRN2", target_bir_lowering=False, debug=False)

    x_in = nc.dram_tensor("x", [B, HI, NI, NI], F32, kind="ExternalInput")
    w_in = nc.dram_tensor("W", [HO, HI, NI, NO], F32, kind="ExternalInput")
    p_out = nc.dram_tensor("P", [B, HO, NO, NO], F32, kind="ExternalOutput")

    with tile.TileContext(nc) as tc, ExitStack() as ctx:
        const = ctx.enter_context(tc.tile_pool(name="const", bufs=1))
        wstage = ctx.enter_context(tc.tile_pool(name="wstage", bufs=1))
        xstage = ctx.enter_context(tc.tile_pool(name="xstage", bufs=4))
        xpool = ctx.enter_context(tc.tile_pool(name="xpool", bufs=2))
        m1pool = ctx.enter_context(tc.tile_pool(name="m1pool", bufs=11))
        ppool = ctx.enter_context(tc.tile_pool(name="ppool", bufs=2))
        m1ps_pool = ctx.enter_context(tc.tile_pool(name="m1ps", bufs=3, space="PSUM"))
        pps_pool = ctx.enter_context(tc.tile_pool(name="pps", bufs=2, space="PSUM"))

        # W resident in SBUF as [j(128), c, o, q]: fp32r for mm1 moving operand,
        # bf16 for mm2 stationary operand. fp32r tiles must be produced by a
        # compute-engine rounding copy (walrus verifier), so DMA into a staging
        # tile and round into dedicated fp32r tiles.
        w_f32r = const.tile([NI, HI, HO, NO], F32R, tag="w_f32r")
        w_bf = const.tile([NI, HI, HO, NO], BF16, tag="w_bf")
        for c in range(HI):
            w_st = wstage.tile([NI, HO, NO], F32, tag="wst", name=f"wst{c}")
            nc.scalar.dma_start(out=w_st[:], in_=w_in[:, c, :, :].transpose([1, 0, 2]))
            nc.vector.tensor_copy(w_bf[:, c], w_st[:])
            nc.vector.tensor_copy(w_f32r[:, c], w_st[:])

        def emit_body():
            emit_groups(nc, tc, x_in, p_out, w_f32r, w_bf, xstage, xpool, m1pool, ppool, m1ps_pool, pps_pool)

        if loop_iters > 1:
            ET = mybir.EngineType
            with tc.For_i(0, loop_iters, 1, hint_engines=(ET.PE, ET.DVE, ET.Activation, ET.SP)):
                emit_body()
        else:
            emit_body()
    nc.finalize()
    return nc


def emit_groups(nc, tc, x_in, p_out, w_f32r, w_bf, xstage, xpool, m1pool, ppool, m1ps_pool, pps_pool):
        def mm2(pps_t, c, o, ph, m1_c):
            nc.tensor.matmul(
                pps_t[ph * 64 : (ph + 1) * 64, :],
                w_bf[:, c, o, :],
                m1_c[:, :, o * 64 : (o + 1) * 64],
                start=(c == 0),
                stop=(c == HI - 1),
                tile_position=(0, ph * 64),
                skip_group_check=True,
            )

        def evict_wave(g, wv, pps, b0):
            # 2 banks -> one SBUF tile -> one DMA per batch for o-pairs 2wv,2wv+1
            p_sb = ppool.tile([NI, 2, BG, NO], F32, tag="psb", name=f"psb_g{g}w{wv}")
            for t in range(2):
                nc.scalar.copy(p_sb[:, t, :, :], pps[t][:])
            for b in range(BG):
                nc.gpsimd.dma_start(
                    out=p_out[b0 + b, 4 * wv : 4 * wv + 4].rearrange(
                        "(t ph) p q -> ph p t q", ph=2
                    ),
                    in_=p_sb[:, :, b, :],
                )

        # x tile: [i(128), b, c, j]; by symmetry also usable as [j, b, c, i].
        # DMAs are emitted one group ahead so they sit ahead of the P-output
        # DMAs in the SP queue (avoids head-of-line blocking).
        def x_load(g):
            # per-batch DMA + round so mm1 can start after 1/4 of the transfer
            x_t = xpool.tile([NI, BG, HI, NI], F32R, tag="xt", name=f"xt{g}")
            for b in range(BG):
                x_sb = xstage.tile([NI, HI, NI], F32, tag="xst", name=f"xst{g}b{b}")
                nc.sync.dma_start(
                    out=x_sb[:], in_=x_in[g * BG + b].transpose([1, 0, 2])
                )
                nc.gpsimd.tensor_copy(x_t[:, b], x_sb[:])
            return x_t

        x_tiles = {0: x_load(0)}

        for g in range(NG):
            b0 = g * BG
            if g + 1 < NG:
                x_tiles[g + 1] = x_load(g + 1)
            x_t = x_tiles.pop(g)

            # wave A (o-pairs 0,1) PSUM accumulators, held across the c loop
            ppsA = [
                pps_pool.tile([NI, BG * NO], F32, tag="pps", name=f"ppsA_g{g}t{t}")
                for t in range(2)
            ]

            # mm1 + eviction + wave-A mm2, c-granular so everything pipelines
            m1_tiles = []
            for c in range(HI):
                m1_c = m1pool.tile([NI, BG, OQ], BF16, tag="m1")
                m1_tiles.append(m1_c)
                for b in range(BG):
                    m1_ps = m1ps_pool.tile([NI, OQ], F32, tag="m1ps")
                    lhsT = x_t[:, b, c, :]
                    for h in range(2):
                        nc.tensor.matmul(
                            m1_ps[:, h * 512 : (h + 1) * 512],
                            lhsT,
                            w_f32r[:, c, h * 8 : (h + 1) * 8, :],
                            start=True,
                            stop=True,
                        )
                    if (c * BG + b) % 2 == 0:
                        nc.vector.tensor_copy(m1_c[:, b, :], m1_ps[:, :])
                    else:
                        nc.scalar.copy(m1_c[:, b, :], m1_ps[:, :])
                    # software pipelining: wave-A mm2 of the PREVIOUS c,
                    # interleaved between mm1 pairs to fill eviction latency
                    if c > 0 and b < 2:
                        t = b
                        for ph in range(2):
                            mm2(ppsA[t], c - 1, 2 * t + ph, ph, m1_tiles[c - 1])
            for t in range(2):
                for ph in range(2):
                    mm2(ppsA[t], HI - 1, 2 * t + ph, ph, m1_tiles[HI - 1])

            evict_wave(g, 0, ppsA, b0)

            # post waves (o-pairs 2..7): o-outer / c-inner dense PE tails
            for wv in range(3):
                ppsB = [
                    pps_pool.tile([NI, BG * NO], F32, tag="pps", name=f"ppsB_g{g}w{wv}t{t}")
                    for t in range(2)
                ]
                for t in range(2):
                    wp = 2 + wv * 2 + t
                    for c in range(HI):
                        for ph in range(2):
                            mm2(ppsB[t], c, 2 * wp + ph, ph, m1_tiles[c])
                evict_wave(g, 1 + wv, ppsB, b0)


def kernel(x: np.ndarray, W: np.ndarray) -> np.ndarray:
    from concourse.bass_utils import run_bass_kernel_spmd

    x = np.ascontiguousarray(x, dtype=np.float32)
    W = np.ascontiguousarray(W, dtype=np.float32)

    if "nc" not in _NC_CACHE:
        _NC_CACHE["nc"] = build_nc()
    nc = _NC_CACHE["nc"]

    in_maps = [
        {"x": x[i * B : (i + 1) * B], "W": W} for i in range(NCORES)
    ]
    res = run_bass_kernel_spmd(nc, in_maps, list(range(NCORES)))
    out = np.concatenate([res.results[i]["P"] for i in range(NCORES)], axis=0)
    return out



# revision 3
# speedup vs baseline: 1.1283x; 1.1283x over previous
"""BiMapGeo forward on 8 NeuronCores (TRN2, Bass/Tile).

P[b,o] = sum_c W[o,c]^T @ x[b,c] @ W[o,c]
  x: (256, 8, 128, 128) fp32 (symmetric in last two dims)
  W: (16, 8, 128, 64) fp32
  P: (256, 16, 64, 64) fp32

Sharding: data-parallel over batch (32 per core), W replicated.

Per-core kernel, per group of BG=4 batches (all-bf16 PE path):
  x load: SWDGE cast-DMA fp32->bf16, [i,(b,c,j)] (x symmetric so [i,j]~[j,i])
  mm1: M1[b,c] = x[b,c] @ W_all[c]   (bf16, stationary=x, moving=W 2x512)
  evict: PSUM fp32 -> SBUF bf16      (DVE / ACT split)
  mm2: P[b,o] += W[o,c]^T @ M1[b,c,o]  (bf16, col-tiled o-pairs run
       concurrently in the PE array; c accumulated in PSUM across the
       full group, 4 banks hold all 16 o)
  evict P -> SBUF -> DMA out
"""

import numpy as np
from contextlib import ExitStack

import concourse.bacc as bacc
import concourse.tile as tile
from concourse import mybir

B_TOT, HI, HO, NI, NO = 256, 8, 16, 128, 64
NCORES = 8
B = B_TOT // NCORES  # 32 batches per core
BG = 4               # batches per group
NG = B // BG         # 8 groups
OQ = HO * NO         # 1024

F32 = mybir.dt.float32
BF16 = mybir.dt.bfloat16

_NC_CACHE = {}


def build_nc(loop_iters: int = 1):
    nc = bacc.Bacc("TRN2", target_bir_lowering=False, debug=False)

    x_in = nc.dram_tensor("x", [B, HI, NI, NI], F32, kind="ExternalInput")
    w_in = nc.dram_tensor("W", [HO, HI, NI, NO], F32, kind="ExternalInput")
    p_out = nc.dram_tensor("P", [B, HO, NO, NO], F32, kind="ExternalOutput")

    with tile.TileContext(nc) as tc, ExitStack() as ctx:
        const = ctx.enter_context(tc.tile_pool(name="const", bufs=1))
        wstage = ctx.enter_context(tc.tile_pool(name="wstage", bufs=2))
        xpool = ctx.enter_context(tc.tile_pool(name="xpool", bufs=3))
        m1pool = ctx.enter_context(tc.tile_pool(name="m1pool", bufs=3))
        ppool = ctx.enter_context(tc.tile_pool(name="ppool", bufs=4))
        m1ps_pool = ctx.enter_context(tc.tile_pool(name="m1ps", bufs=2, space="PSUM"))
        pps_pool = ctx.enter_context(tc.tile_pool(name="pps", bufs=4, space="PSUM"))

        # W resident in SBUF as [j(128), c, o, q] bf16 (mm1 moving operand and
        # mm2 stationary operand).
        w_bf = const.tile([NI, HI, HO, NO], BF16, tag="w_bf")
        for c in range(HI):
            w_st = wstage.tile([NI, HO, NO], F32, tag="wst", name=f"wst{c}")
            nc.sync.dma_start(out=w_st[:], in_=w_in[:, c, :, :].transpose([1, 0, 2]))
            nc.vector.tensor_copy(w_bf[:, c], w_st[:])

        def emit_body():
            emit_groups(nc, tc, x_in, p_out, w_bf, xpool, m1pool, ppool,
                        m1ps_pool, pps_pool)

        if loop_iters > 1:
            ET = mybir.EngineType
            with tc.For_i(0, loop_iters, 1, hint_engines=(ET.PE, ET.DVE, ET.Activation, ET.SP)):
                emit_body()
        else:
            emit_body()
    nc.finalize()
    return nc


def emit_groups(nc, tc, x_in, p_out, w_bf, xpool, m1pool, ppool, m1ps_pool, pps_pool):
    # x tile: [i(128), b, c, j] bf16; by symmetry also usable as [j, b, c, i].
    # SWDGE DMA casts fp32 -> bf16 in flight.
    def x_load(g):
        x_t = xpool.tile([NI, BG, HI, NI], BF16, tag="xt", name=f"xt{g}")
        for b in range(BG):
            nc.gpsimd.dma_start(
                out=x_t[:, b], in_=x_in[g * BG + b].transpose([1, 0, 2])
            )
        return x_t

    def mm2_c(c, pps, m1_tiles):
        # all 16 o for channel c; (ph) pairs are col-tiled and run
        # concurrently in the PE array
        for t in range(4):
            for s in range(2):
                for ph in range(2):
                    o = 4 * t + 2 * s + ph
                    # start=True clears has_written for the whole bank row
                    # (all 512 cols) in the written partitions, so only the
                    # first stream per (bank, partition-half) may set it; the
                    # s=1 stream's first write lands on cleared bits and
                    # overwrites per-element.
                    nc.tensor.matmul(
                        pps[t][ph * 64 : (ph + 1) * 64, s],
                        w_bf[:, c, o, :],
                        m1_tiles[c][:, :, o * 64 : (o + 1) * 64],
                        start=(c == 0 and s == 0),
                        stop=(c == HI - 1),
                        tile_position=(0, ph * 64),
                        skip_group_check=True,
                    )

    x_tiles = {0: x_load(0)}

    for g in range(NG):
        b0 = g * BG
        if g + 1 < NG:
            x_tiles[g + 1] = x_load(g + 1)
        x_t = x_tiles.pop(g)

        # 4 PSUM banks hold P accumulators for all 16 o across the c loop
        pps = [
            pps_pool.tile([NI, 2, BG, NO], F32, tag="pps", name=f"pps_g{g}t{t}")
            for t in range(4)
        ]

        m1_tiles = []
        for c in range(HI):
            m1_c = m1pool.tile([NI, BG, OQ], BF16, tag="m1")
            m1_tiles.append(m1_c)
            for b in range(BG):
                m1_ps = m1ps_pool.tile([NI, OQ], F32, tag="m1ps")
                lhsT = x_t[:, b, c, :]
                for h in range(2):
                    nc.tensor.matmul(
                        m1_ps[:, h * 512 : (h + 1) * 512],
                        lhsT,
                        w_bf[:, c, h * 8 : (h + 1) * 8, :],
                        start=True,
                        stop=True,
                    )
                # M1 eviction fp32->bf16, split DVE (14/32) vs ACT (18/32)
                if (c * BG + b) % 16 < 7:
                    nc.vector.tensor_copy(m1_c[:, b, :], m1_ps[:, :])
                else:
                    nc.scalar.copy(m1_c[:, b, :], m1_ps[:, :])
            # software pipelining: mm2 of the previous c fills eviction latency
            if c > 0:
                mm2_c(c - 1, pps, m1_tiles)
        mm2_c(HI - 1, pps, m1_tiles)

        # P eviction: bank -> SBUF -> one DMA per batch per bank
        for t in range(4):
            p_sb = ppool.tile([NI, 2, BG, NO], F32, tag="psb", name=f"psb_g{g}t{t}")
            if t % 2 == 0:
                nc.vector.tensor_copy(p_sb[:], pps[t][:])
            else:
                nc.scalar.copy(p_sb[:], pps[t][:])
            for b in range(BG):
                nc.gpsimd.dma_start(
                    out=p_out[b0 + b, 4 * t : 4 * t + 4].rearrange(
                        "(s ph) p q -> (ph p) s q", ph=2
                    ),
                    in_=p_sb[:, :, b, :],
                )


def kernel(x: np.ndarray, W: np.ndarray) -> np.ndarray:
    from concourse.bass_utils import run_bass_kernel_spmd

    x = np.ascontiguousarray(x, dtype=np.float32)
    W = np.ascontiguousarray(W, dtype=np.float32)

    if "nc" not in _NC_CACHE:
        _NC_CACHE["nc"] = build_nc()
    nc = _NC_CACHE["nc"]

    in_maps = [
        {"x": x[i * B : (i + 1) * B], "W": W} for i in range(NCORES)
    ]
    res = run_bass_kernel_spmd(nc, in_maps, list(range(NCORES)))
    out = np.concatenate([res.results[i]["P"] for i in range(NCORES)], axis=0)
    return out


# revision 37
# speedup vs baseline: 1.6365x; 1.4505x over previous
"""BiMapGeo forward on 8 NeuronCores (TRN2, Bass/Tile).

P[b,o] = sum_c W[o,c]^T @ x[b,c] @ W[o,c]
  x: (256, 8, 128, 128) fp32 (symmetric in last two dims)
  W: (16, 8, 128, 64) fp32
  P: (256, 16, 64, 64) fp32

Sharding: data-parallel over batch (32 per core), W replicated.

Per-core kernel, per group of BG=4 batches (all-bf16 PE path):
  x load: SWDGE cast-DMA fp32->bf16, [i,(b,c,j)] (x symmetric so [i,j]~[j,i])
  mm1: M1[b,c] = x[b,c] @ W_all[c]   (bf16, stationary=x, moving=W 2x512,
       PSUM pipeline depth 3)
  evict: PSUM fp32 -> SBUF bf16      (DVE low half / ACT high half, parallel
       on different PSUM banks)
  mm2: P[b,o] += W[o,c]^T @ M1[b,c,o]  (bf16, two waves of 8 o; wave A
       interleaved with mm1 per-c, wave B after; 2 PSUM banks per wave,
       col-tiled 64-partition halves share a bank)
  evict P -> SBUF -> DMA out
"""

import numpy as np
from contextlib import ExitStack

import concourse.bacc as bacc
import concourse.tile as tile
from concourse import mybir

B_TOT, HI, HO, NI, NO = 256, 8, 16, 128, 64
NCORES = 8
B = B_TOT // NCORES  # 32 batches per core
BG = 4               # batches per group
NG = B // BG         # 8 groups
OQ = HO * NO         # 1024

F32 = mybir.dt.float32
BF16 = mybir.dt.bfloat16

_NC_CACHE = {}

# tunables (A/B tested on HW; alt/noPool won the interleaved comparison)
EVICT_MODE = "alt"    # "half": DVE+ACT parallel halves; "alt": alternate whole
HINT_POOL = False     # include Pool in For_i hint engines


def build_nc(loop_iters: int = 1, ablate: str = "", timing_io: bool = False):
    nc = bacc.Bacc("TRN2", target_bir_lowering=False, debug=False)

    if timing_io:
        # timing-only build: x/P live in device DRAM (garbage data, same
        # addresses and DMA traffic) so each run ships almost nothing over
        # the axon tunnel. W stays an input; a tiny dummy output remains.
        x_in = nc.dram_tensor("x", [B, HI, NI, NI], F32, kind="Internal")
        w_in = nc.dram_tensor("W", [HO, HI, NI, NO], F32, kind="ExternalInput")
        p_out = nc.dram_tensor("P", [B, HO, NO, NO], F32, kind="Internal")
        dummy = nc.dram_tensor("out", [NI, 4], F32, kind="ExternalOutput")
    else:
        x_in = nc.dram_tensor("x", [B, HI, NI, NI], F32, kind="ExternalInput")
        w_in = nc.dram_tensor("W", [HO, HI, NI, NO], F32, kind="ExternalInput")
        p_out = nc.dram_tensor("P", [B, HO, NO, NO], F32, kind="ExternalOutput")
        dummy = None

    with tile.TileContext(nc) as tc, ExitStack() as ctx:
        const = ctx.enter_context(tc.tile_pool(name="const", bufs=1))
        wstage = ctx.enter_context(tc.tile_pool(name="wstage", bufs=2))
        xpool = ctx.enter_context(tc.tile_pool(name="xpool", bufs=3))
        m1pool = ctx.enter_context(tc.tile_pool(name="m1pool", bufs=10))
        ppool = ctx.enter_context(tc.tile_pool(name="ppool", bufs=2))
        m1s_pool = ctx.enter_context(tc.tile_pool(name="m1s", bufs=1))
        m1ps_pool = ctx.enter_context(tc.tile_pool(name="m1ps", bufs=3, space="PSUM"))
        pps_pool = ctx.enter_context(tc.tile_pool(name="pps", bufs=1, space="PSUM"))

        # W resident in SBUF as [j(128), c, o, q] bf16 (mm1 moving operand and
        # mm2 stationary operand).
        w_bf = const.tile([NI, HI, HO, NO], BF16, tag="w_bf")
        for c in range(HI):
            w_st = wstage.tile([NI, HO, NO], F32, tag="wst", name=f"wst{c}")
            nc.sync.dma_start(out=w_st[:], in_=w_in[:, c, :, :].transpose([1, 0, 2]))
            nc.vector.tensor_copy(w_bf[:, c], w_st[:])

        def emit_body():
            emit_groups(nc, tc, x_in, p_out, w_bf, xpool, m1pool, ppool,
                        m1ps_pool, pps_pool, ablate, m1s_pool)

        if loop_iters > 1:
            ET = mybir.EngineType
            hints = (ET.PE, ET.DVE, ET.Activation, ET.SP)
            if HINT_POOL:
                hints = hints + (ET.Pool,)
            with tc.For_i(0, loop_iters, 1, hint_engines=hints):
                emit_body()
        else:
            emit_body()
        if dummy is not None:
            d_sb = wstage.tile([NI, 4], F32, tag="dummy")
            nc.vector.memset(d_sb[:], 0.0)
            nc.sync.dma_start(out=dummy[:], in_=d_sb[:])
    nc.finalize()
    return nc


def emit_groups(nc, tc, x_in, p_out, w_bf, xpool, m1pool, ppool, m1ps_pool, pps_pool,
                ablate="", m1s_pool=None):
    # ablate: "" full | "x" x loads only | "xsync" HWDGE fp32 x loads |
    #         "mm1" x+mm1 only | "mm1evict" x+mm1+evict |
    #         "mm2" mm2+Pout only (garbage m1) | "nomm2" all but mm2
    if ablate == "none":
        return
    if ablate in ("mm1pure", "mm1split", "mm1split32"):
        # pure LDW+MM stream: static x, no DMAs, no evictions, no mm2
        xs = m1s_pool.tile([NI, BG, HI, NI], BF16, tag="xs", name="xs")
        nc.vector.memset(xs[:], 0.25)
        for g in range(NG):
            for c in range(HI):
                for b in range(BG):
                    m1_ps = m1ps_pool.tile([NI, OQ], F32, tag="m1ps")
                    for h in range(2):
                        if ablate in ("mm1split", "mm1split32"):
                            # stationary split into M=64/M=32 col tiles
                            # sharing one W stream; LDWs alternate col
                            # groups so they pull ahead
                            nt = 2 if ablate == "mm1split" else 4
                            mw = 128 // nt
                            for lf in range(nt):
                                nc.tensor.matmul(
                                    m1_ps[lf * mw : (lf + 1) * mw,
                                          h * 512 : (h + 1) * 512],
                                    xs[:, b, c, lf * mw : (lf + 1) * mw],
                                    w_bf[:, c, h * 8 : (h + 1) * 8, :],
                                    start=True,
                                    stop=True,
                                    tile_position=(0, lf * mw),
                                    skip_group_check=True,
                                )
                        else:
                            nc.tensor.matmul(
                                m1_ps[:, h * 512 : (h + 1) * 512],
                                xs[:, b, c, :],
                                w_bf[:, c, h * 8 : (h + 1) * 8, :],
                                start=True,
                                stop=True,
                            )
        return
    do_mm1 = ablate in ("", "mm1", "mm1evict", "nomm2")
    do_x = do_mm1 or ablate in ("x", "xsync")
    do_evict = ablate in ("", "mm1evict", "nomm2")
    do_mm2 = ablate in ("", "mm2")
    do_pout = ablate in ("", "mm2", "nomm2")

    # x tile: [i(128), b, c, j] bf16; by symmetry also usable as [j, b, c, i].
    # SWDGE DMA casts fp32 -> bf16 in flight.
    def x_load(g):
        if ablate == "xsync":
            x_t = xpool.tile([NI, BG, HI, NI], F32, tag="xt", name=f"xt{g}")
            for b in range(BG):
                nc.sync.dma_start(
                    out=x_t[:, b], in_=x_in[g * BG + b].transpose([1, 0, 2])
                )
            return x_t
        x_t = xpool.tile([NI, BG, HI, NI], BF16, tag="xt", name=f"xt{g}")
        for b in range(BG):
            nc.gpsimd.dma_start(
                out=x_t[:, b], in_=x_in[g * BG + b].transpose([1, 0, 2])
            )
        return x_t

    def mm2_wave(c, wv, pps_w, m1_tiles):
        # 8 o's of wave wv for channel c.
        # pps_w: [128, 2(pr bank), 2(s), BG, 64]; o = 8*wv + 4*pr + 2*s + ph.
        # start=True clears has_written for the whole bank row in the written
        # partitions, so only the first (c==0, s==0) write per
        # (bank, partition-half) sets it.
        for pr in range(2):
            for s in range(2):
                for ph in range(2):
                    o = 8 * wv + 4 * pr + 2 * s + ph
                    nc.tensor.matmul(
                        pps_w[ph * 64 : (ph + 1) * 64, pr, s],
                        w_bf[:, c, o, :],
                        m1_tiles[c][:, :, o * 64 : (o + 1) * 64],
                        start=(c == 0 and s == 0),
                        stop=(c == HI - 1),
                        tile_position=(0, ph * 64),
                        skip_group_check=True,
                    )

    def evict_wave(g, wv, pps_w, b0):
        # 2 banks -> SBUF (DVE bank 0, ACT bank 1) -> one DMA per batch
        p_sb = ppool.tile([NI, 2, 2, BG, NO], F32, tag="psb", name=f"psb_g{g}w{wv}")
        nc.vector.tensor_copy(p_sb[:, 0], pps_w[:, 0])
        nc.scalar.copy(p_sb[:, 1], pps_w[:, 1])
        for b in range(BG):
            # HWDGE: no cast needed, keeps Q7/SWDGE free for the x cast-DMAs
            nc.sync.dma_start(
                out=p_out[b0 + b, 8 * wv : 8 * wv + 8].rearrange(
                    "(pr s ph) p q -> (ph p) pr s q", pr=2, s=2, ph=2
                ),
                in_=p_sb[:, :, :, b, :],
            )

    x_tiles = {0: x_load(0)} if do_x else {}

    static_m1 = None
    if ablate == "mm2":
        sm1 = m1s_pool.tile([NI, BG, OQ], BF16, tag="sm1", name="sm1")
        nc.vector.memset(sm1[:], 0.25)
        static_m1 = [sm1] * HI

    for g in range(NG):
        b0 = g * BG
        if do_x:
            if g + 1 < NG:
                x_tiles[g + 1] = x_load(g + 1)
            x_t = x_tiles.pop(g)

        # wave A accumulators: 2 PSUM banks hold o 0..7 across the c loop
        pps_a = pps_pool.tile([NI, 2, 2, BG, NO], F32, tag="pps",
                              name=f"ppsA_g{g}") if (do_mm2 or do_pout) else None

        m1_tiles = static_m1 if static_m1 is not None else []
        for c in range(HI):
            if do_mm1:
                m1_c = m1pool.tile([NI, BG, OQ], BF16, tag="m1")
                m1_tiles.append(m1_c)
                for b in range(BG):
                    m1_ps = m1ps_pool.tile([NI, OQ], F32, tag="m1ps")
                    lhsT = x_t[:, b, c, :]
                    for h in range(2):
                        nc.tensor.matmul(
                            m1_ps[:, h * 512 : (h + 1) * 512],
                            lhsT,
                            w_bf[:, c, h * 8 : (h + 1) * 8, :],
                            start=True,
                            stop=True,
                        )
                    if do_evict:
                        if EVICT_MODE == "half":
                            # M1 eviction fp32->bf16: DVE and ACT in parallel
                            # on different PSUM banks to halve latency
                            nc.vector.tensor_copy(m1_c[:, b, :512], m1_ps[:, :512])
                            nc.scalar.copy(m1_c[:, b, 512:], m1_ps[:, 512:])
                        else:
                            if (c * BG + b) % 2 == 0:
                                nc.vector.tensor_copy(m1_c[:, b, :], m1_ps[:, :])
                            else:
                                nc.scalar.copy(m1_c[:, b, :], m1_ps[:, :])
            # software pipelining: wave-A mm2 of the previous c fills
            # eviction latency
            if do_mm2 and c > 0:
                mm2_wave(c - 1, 0, pps_a, m1_tiles)
        if do_mm2:
            mm2_wave(HI - 1, 0, pps_a, m1_tiles)
        if do_pout:
            evict_wave(g, 0, pps_a, b0)

        # wave B: o 8..15, dense c-scan over the retained m1 tiles,
        # reusing the same 2 PSUM banks (bufs=1)
        if do_mm2 or do_pout:
            pps_b = pps_pool.tile([NI, 2, 2, BG, NO], F32, tag="pps",
                                  name=f"ppsB_g{g}")
            if do_mm2:
                for c in range(HI):
                    mm2_wave(c, 1, pps_b, m1_tiles)
            if do_pout:
                evict_wave(g, 1, pps_b, b0)


def kernel(x: np.ndarray, W: np.ndarray) -> np.ndarray:
    from concourse.bass_utils import run_bass_kernel_spmd

    x = np.ascontiguousarray(x, dtype=np.float32)
    W = np.ascontiguousarray(W, dtype=np.float32)

    if "nc" not in _NC_CACHE:
        _NC_CACHE["nc"] = build_nc()
    nc = _NC_CACHE["nc"]

    in_maps = [
        {"x": x[i * B : (i + 1) * B], "W": W} for i in range(NCORES)
    ]
    res = run_bass_kernel_spmd(nc, in_maps, list(range(NCORES)))
    out = np.concatenate([res.results[i]["P"] for i in range(NCORES)], axis=0)
    return out
